# revision 28
# baseline (speedup 1.0000x reference)
"""Trainium2 Bass kernel for EquivLayerNorm (segment_reduce).

Contract: kernel(**inputs) takes FULL unsharded inputs (as produced by
setup_inputs()) and returns the full (sout, vout) tuple. Internally the
nodes are sharded across 8 NeuronCores by contiguous segment-id ranges
(core c owns segments [c*128, (c+1)*128)), one SPMD NEFF runs on all 8
cores, and the per-core outputs are concatenated.

Math (identical to the reference up to fp rounding):
  per seg g:  smean_g = sum_{i in g} sum_f s[i,f] / (128*cnt_g)
              var_g   = max( sum s^2/(128*cnt) - smean^2, eps )
              vmean_g = max( sum v^2/(64*cnt), eps )
  sout = (s - smean)/var * weight_s + bias_s     (divide by var, not sqrt)
  vout = v / vmean

Single-read schedule: node data is loaded once; normalization of chunk c
runs after the stats pass of chunk c+1, at which point every segment
touching chunk c is complete (a segment spans < K_CH*128 nodes), so the
still-accumulating PSUM stats are already final for those segments.
"""

import os
import numpy as np

N_CORES = 8
B_TOTAL = 1024
B_LOCAL = B_TOTAL // N_CORES  # 128 segments per core
SDIM = 128
VROWS = 3
VDIM = 64
VF = VROWS * VDIM  # 192
EPS = 1e-6
K_CH = 8     # node-tiles per DMA chunk; also the pass-B lag unit
GHOST = 999.0

_BUILD_CACHE = {}


def _build(T: int, has_affine: bool, lag: int = 1):
    """Build + compile the SPMD Bass program for T node-tiles per core.

    lag: pass-B of chunk c runs after pass-A of chunk c+lag; must satisfy
    max_segment_nodes <= lag * K_CH * 128 so every segment touching chunk c
    is fully accumulated by then.
    """
    key = (T, has_affine, lag)
    if key in _BUILD_CACHE:
        return _BUILD_CACHE[key]

    from contextlib import ExitStack
    import concourse.bass as bass
    import concourse.tile as tile
    from concourse import bacc, mybir

    f32 = mybir.dt.float32
    assert T % K_CH == 0
    n_ch = T // K_CH
    KW = K_CH * 128  # 1024

    nc = bacc.Bacc(
        "TRN2",
        target_bir_lowering=False,
        debug=False,
        enable_asserts=False,
        num_devices=N_CORES,
    )

    NP = T * 128  # padded nodes per core
    s_in = nc.dram_tensor("s_in", [NP, SDIM], f32, kind="ExternalInput").ap()
    v_in = nc.dram_tensor("v_in", [NP, VF], f32, kind="ExternalInput").ap()
    bt_in = nc.dram_tensor("batch_t", [128, T], f32, kind="ExternalInput").ap()
    br_in = nc.dram_tensor("batch_rows", [n_ch, KW], f32, kind="ExternalInput").ap()
    ic128_in = nc.dram_tensor("invcnt128", [128, 1], f32, kind="ExternalInput").ap()
    ic64_in = nc.dram_tensor("invcnt64", [128, 1], f32, kind="ExternalInput").ap()
    if has_affine:
        w_in = nc.dram_tensor("weight_s", [SDIM], f32, kind="ExternalInput").ap()
        b_in = nc.dram_tensor("bias_s", [SDIM], f32, kind="ExternalInput").ap()
    s_out = nc.dram_tensor("s_out", [NP, SDIM], f32, kind="ExternalOutput").ap()
    v_out = nc.dram_tensor("v_out", [NP, VF], f32, kind="ExternalOutput").ap()

    # node-tile views: [128 part, T, feat]
    s_iv = s_in.rearrange("(t p) f -> p t f", p=128)
    v_iv = v_in.rearrange("(t p) f -> p t f", p=128)
    s_ov = s_out.rearrange("(t p) f -> p t f", p=128)
    v_ov = v_out.rearrange("(t p) f -> p t f", p=128)

    eq = mybir.AluOpType.is_equal
    mul = mybir.AluOpType.mult
    sub = mybir.AluOpType.subtract

    with tile.TileContext(nc) as tc, ExitStack() as ctx:
        singles = ctx.enter_context(tc.tile_pool(name="singles", bufs=1))
        p_s = ctx.enter_context(tc.tile_pool(name="p_s", bufs=lag + 3))
        p_v = ctx.enter_context(tc.tile_pool(name="p_v", bufs=lag + 3))
        p_so = ctx.enter_context(tc.tile_pool(name="p_so", bufs=4))
        p_vo = ctx.enter_context(tc.tile_pool(name="p_vo", bufs=4))
        p_oh = ctx.enter_context(tc.tile_pool(name="p_oh", bufs=6))
        p_oht = ctx.enter_context(tc.tile_pool(name="p_oht", bufs=3))
        p_stage = ctx.enter_context(tc.tile_pool(name="p_stage", bufs=3))
        p_qw = ctx.enter_context(tc.tile_pool(name="p_qw", bufs=8))
        p_pn = ctx.enter_context(tc.tile_pool(name="p_pn", bufs=3))
        p_tab = ctx.enter_context(tc.tile_pool(name="p_tab", bufs=3))
        p_tiny = ctx.enter_context(tc.tile_pool(name="p_tiny", bufs=2))
        ps_acc = ctx.enter_context(tc.tile_pool(name="ps_acc", bufs=1, space="PSUM"))
        ps_bb = ctx.enter_context(tc.tile_pool(name="ps_bb", bufs=2, space="PSUM"))
        ps_pn = ctx.enter_context(tc.tile_pool(name="ps_pn", bufs=2, space="PSUM"))

        # --- constants ---
        iota_row = singles.tile([128, 128], f32)
        nc.gpsimd.iota(iota_row[:], pattern=[[1, 128]], base=0,
                       channel_multiplier=0, allow_small_or_imprecise_dtypes=True)
        iota_col = singles.tile([128, 1], f32)
        nc.gpsimd.iota(iota_col[:], pattern=[[1, 1]], base=0,
                       channel_multiplier=1, allow_small_or_imprecise_dtypes=True)
        ones_row = singles.tile([1, 128], f32)
        nc.vector.memset(ones_row[:], 1.0)

        sb_bt = singles.tile([128, T], f32)
        nc.sync.dma_start(out=sb_bt[:], in_=bt_in[:])
        sb_ic128 = singles.tile([128, 1], f32)
        nc.sync.dma_start(out=sb_ic128[:], in_=ic128_in[:])
        sb_ic64 = singles.tile([128, 1], f32)
        nc.sync.dma_start(out=sb_ic64[:], in_=ic64_in[:])
        if has_affine:
            sb_w = singles.tile([128, SDIM], f32)
            nc.sync.dma_start(out=sb_w[:], in_=bass.AP(
                tensor=w_in.tensor, offset=w_in.offset,
                ap=[[0, 128]] + list(w_in.ap)))
            sb_b = singles.tile([128, SDIM], f32)
            nc.sync.dma_start(out=sb_b[:], in_=bass.AP(
                tensor=b_in.tensor, offset=b_in.offset,
                ap=[[0, 128]] + list(b_in.ap)))

        junk_s = singles.tile([128, 128], f32)
        junk_v = singles.tile([128, VF], f32)

        # per-chunk PSUM groups, drained into SBUF running accumulators
        psum_sf = ps_acc.tile([128, 128], f32)  # per-seg per-feature sum of s
        psum_qw = ps_acc.tile([128, 2], f32)    # per-seg [sum s^2, sum v^2]
        sacc_sf = singles.tile([128, 128], f32)
        sacc_qw = singles.tile([128, 2], f32)
        nc.vector.memset(sacc_sf[:], 0.0)
        nc.vector.memset(sacc_qw[:], 0.0)

        saved = {}

        def pass_a(c):
            sl = slice(c * K_CH, (c + 1) * K_CH)
            s_ch = p_s.tile([128, K_CH, SDIM], f32, tag="s_ch")
            nc.sync.dma_start(out=s_ch[:], in_=s_iv[:, sl, :])
            v_ch = p_v.tile([128, K_CH, VF], f32, tag="v_ch")
            nc.gpsimd.dma_start(out=v_ch[:], in_=v_iv[:, sl, :])
            saved[c] = (s_ch, v_ch)
            for k in range(K_CH):
                t = c * K_CH + k
                onehot = p_oh.tile([128, 128], f32, tag="oh")
                nc.vector.tensor_scalar(
                    out=onehot[:], in0=iota_row[:],
                    scalar1=sb_bt[:, t:t + 1], scalar2=None, op0=eq)
                qw = p_qw.tile([128, 2], f32, tag="qw")
                # sum_f s^2 per node on DVE (junk elementwise out + row accum)
                nc.vector.scalar_tensor_tensor(
                    out=junk_s[:], in0=s_ch[:, k, :], scalar=1.0,
                    in1=s_ch[:, k, :], op0=mul, op1=mul,
                    accum_out=qw[:, 0:1])
                # sum v^2 per node on ACT
                nc.scalar.activation(
                    out=junk_v[:], in_=v_ch[:, k, :],
                    func=mybir.ActivationFunctionType.Square,
                    accum_out=qw[:, 1:2])
                nc.tensor.matmul(psum_sf[:], onehot[:], s_ch[:, k, :],
                                 start=(k == 0), stop=(k == K_CH - 1))
                nc.tensor.matmul(psum_qw[:], onehot[:], qw[:],
                                 start=(k == 0), stop=(k == K_CH - 1))
            nc.vector.tensor_tensor(out=sacc_sf[:], in0=sacc_sf[:],
                                    in1=psum_sf[:], op=mybir.AluOpType.add)
            nc.vector.tensor_tensor(out=sacc_qw[:], in0=sacc_qw[:],
                                    in1=psum_qw[:], op=mybir.AluOpType.add)

        def table_math():
            # Valid rows: segments fully accumulated so far. tabs cols:
            # 0: winv, 1: -smean*winv, 2: vinv (rest zero).
            tabs = p_tab.tile([128, 8], f32, tag="tabs")
            nc.vector.memset(tabs[:, 3:8], 0.0)
            msum = p_tiny.tile([128, 1], f32, tag="msum")
            nc.vector.reduce_sum(out=msum[:], in_=sacc_sf[:],
                                 axis=mybir.AxisListType.X)
            smean = p_tiny.tile([128, 1], f32, tag="smean")
            nc.vector.tensor_tensor(out=smean[:], in0=msum[:],
                                    in1=sb_ic128[:], op=mul)
            qmean = p_tiny.tile([128, 1], f32, tag="qmean")
            nc.vector.tensor_tensor(out=qmean[:], in0=sacc_qw[:, 0:1],
                                    in1=sb_ic128[:], op=mul)
            wmean = p_tiny.tile([128, 1], f32, tag="wmean")
            nc.vector.tensor_tensor(out=wmean[:], in0=sacc_qw[:, 1:2],
                                    in1=sb_ic64[:], op=mul)
            var = p_tiny.tile([128, 1], f32, tag="var")
            # var = qmean - smean^2  via (smean * -smean) + qmean
            nc.vector.scalar_tensor_tensor(
                out=var[:], in0=smean[:], scalar=-1.0, in1=smean[:],
                op0=mul, op1=mul)
            nc.vector.tensor_tensor(out=var[:], in0=var[:], in1=qmean[:],
                                    op=mybir.AluOpType.add)
            nc.vector.tensor_scalar_max(out=var[:], in0=var[:], scalar1=EPS)
            nc.vector.reciprocal(out=tabs[:, 0:1], in_=var[:])
            # -smean * winv
            nc.vector.scalar_tensor_tensor(
                out=tabs[:, 1:2], in0=smean[:], scalar=-1.0,
                in1=tabs[:, 0:1], op0=mul, op1=mul)
            nc.vector.tensor_scalar_max(out=wmean[:], in0=wmean[:], scalar1=EPS)
            nc.vector.reciprocal(out=tabs[:, 2:3], in_=wmean[:])
            return tabs

        def pass_b(c, tabs):
            sl = slice(c * K_CH, (c + 1) * K_CH)
            s_ch, v_ch = saved.pop(c)
            stage = p_stage.tile([1, KW], f32, tag="stage")
            nc.gpsimd.dma_start(out=stage[:], in_=br_in[c:c + 1, :])
            bb = ps_bb.tile([128, KW], f32, tag="bb")
            nc.tensor.matmul(bb[:, 0:512], ones_row[:], stage[:, 0:512],
                             start=True, stop=True)
            nc.tensor.matmul(bb[:, 512:KW], ones_row[:], stage[:, 512:KW],
                             start=True, stop=True)
            oht = p_oht.tile([128, KW], f32, tag="oht")
            nc.vector.tensor_scalar(
                out=oht[:], in0=bb[:],
                scalar1=iota_col[:], scalar2=None, op0=eq)
            pn8 = ps_pn.tile([128, K_CH, 8], f32, tag="pn8")
            for k in range(K_CH):
                nc.tensor.matmul(pn8[:, k, :], oht[:, k * 128:(k + 1) * 128],
                                 tabs[:], start=True, stop=True)
            pn = p_pn.tile([128, K_CH, 8], f32, tag="pn")
            nc.vector.tensor_copy(out=pn[:], in_=pn8[:])
            s_o = p_so.tile([128, K_CH, SDIM], f32, tag="s_o")
            v_o = p_vo.tile([128, K_CH, VF], f32, tag="v_o")
            for k in range(K_CH):
                # sout = s*winv + (-smean*winv); alternate ACT/DVE for balance
                if k % 2 == 0:
                    nc.scalar.activation(
                        out=s_o[:, k, :], in_=s_ch[:, k, :],
                        func=mybir.ActivationFunctionType.Identity,
                        bias=pn[:, k, 1:2], scale=pn[:, k, 0:1])
                else:
                    nc.vector.tensor_scalar(
                        out=s_o[:, k, :], in0=s_ch[:, k, :],
                        scalar1=pn[:, k, 0:1], scalar2=pn[:, k, 1:2],
                        op0=mul, op1=mybir.AluOpType.add)
                if has_affine:
                    nc.vector.scalar_tensor_tensor(
                        out=s_o[:, k, :], in0=s_o[:, k, :], scalar=1.0,
                        in1=sb_w[:], op0=mul, op1=mul)
                    nc.vector.tensor_tensor(out=s_o[:, k, :], in0=s_o[:, k, :],
                                            in1=sb_b[:], op=mybir.AluOpType.add)
                nc.vector.tensor_scalar(
                    out=v_o[:, k, :], in0=v_ch[:, k, :],
                    scalar1=pn[:, k, 2:3], scalar2=None, op0=mul)
            nc.sync.dma_start(out=s_ov[:, sl, :], in_=s_o[:])
            nc.gpsimd.dma_start(out=v_ov[:, sl, :], in_=v_o[:])

        for c in range(n_ch):
            pass_a(c)
            if c >= lag:
                pass_b(c - lag, table_math())
        tabs_final = table_math()
        for c in range(max(0, n_ch - lag), n_ch):
            pass_b(c, tabs_final)

    nc.compile()
    _BUILD_CACHE[key] = nc
    return nc


def _prep(s, v, batch, weight_s, bias_s):
    """Shard + pad full inputs into 8 per-core input maps."""
    N = s.shape[0]
    batch = np.asarray(batch).astype(np.int64)
    edges = np.arange(0, B_TOTAL + 1, B_LOCAL, dtype=np.int64)
    bounds = np.searchsorted(batch, edges, side="left")
    counts_per_core = np.diff(bounds)
    cnt_seg = np.bincount(batch, minlength=B_TOTAL).astype(np.int64)

    T = int(np.ceil(counts_per_core.max() / 128))
    T = ((T + K_CH - 1) // K_CH) * K_CH
    NP = T * 128
    max_seg = int(cnt_seg.max())
    lag = max(1, -(-max_seg // (K_CH * 128)))

    v2 = np.ascontiguousarray(v.reshape(N, VF))
    in_maps = []
    for c in range(N_CORES):
        lo, hi = int(bounds[c]), int(bounds[c + 1])
        n_c = hi - lo
        s_pad = np.zeros((NP, SDIM), np.float32)
        s_pad[:n_c] = s[lo:hi]
        v_pad = np.zeros((NP, VF), np.float32)
        v_pad[:n_c] = v2[lo:hi]
        loc = np.full(NP, GHOST, np.float32)
        loc[:n_c] = (batch[lo:hi] - c * B_LOCAL).astype(np.float32)
        bt = np.ascontiguousarray(loc.reshape(T, 128).T)  # [128, T]
        br = np.ascontiguousarray(loc.reshape(T // K_CH, K_CH * 128))
        cs = cnt_seg[c * B_LOCAL:(c + 1) * B_LOCAL].astype(np.float64)
        csm = np.maximum(cs, 1.0)
        ic128 = (1.0 / (128.0 * csm)).astype(np.float32).reshape(128, 1)
        ic64 = (1.0 / (64.0 * csm)).astype(np.float32).reshape(128, 1)
        m = {
            "s_in": s_pad, "v_in": v_pad, "batch_t": bt, "batch_rows": br,
            "invcnt128": ic128, "invcnt64": ic64,
        }
        in_maps.append(m)
    return in_maps, bounds, T, lag


def kernel(s, v, weight_s, bias_s, batch, num_segments, **kwargs):
    import concourse.bass_utils as bass_utils

    s = np.asarray(s, dtype=np.float32)
    v = np.asarray(v, dtype=np.float32)
    weight_s = np.asarray(weight_s, dtype=np.float32)
    bias_s = np.asarray(bias_s, dtype=np.float32)
    assert int(num_segments) == B_TOTAL
    N = s.shape[0]

    batch_arr = np.asarray(batch)
    order = None
    if np.any(np.diff(batch_arr) < 0):  # defensive: reference data is sorted
        order = np.argsort(batch_arr, kind="stable")
        batch_arr = batch_arr[order]
        s = s[order]
        v = v[order]

    has_affine = not (np.all(weight_s == 1.0) and np.all(bias_s == 0.0))
    in_maps, bounds, T, lag = _prep(s, v, batch_arr, weight_s, bias_s)
    if has_affine:
        for m in in_maps:
            m["weight_s"] = weight_s
            m["bias_s"] = bias_s

    nc = _build(T, has_affine, lag)
    res = bass_utils.run_bass_kernel_spmd(
        nc, in_maps, core_ids=list(range(N_CORES)),
        trace=bool(int(os.environ.get("KERNEL_TRACE", "0"))),
    )
    kernel.last_results = res

    sout = np.empty((N, SDIM), np.float32)
    vout = np.empty((N, VF), np.float32)
    for c in range(N_CORES):
        lo, hi = int(bounds[c]), int(bounds[c + 1])
        n_c = hi - lo
        sout[lo:hi] = res.results[c]["s_out"][:n_c]
        vout[lo:hi] = res.results[c]["v_out"][:n_c]
    if order is not None:
        inv = np.empty_like(order)
        inv[order] = np.arange(N)
        sout = sout[inv]
        vout = vout[inv]
    return sout, vout.reshape(N, VROWS, VDIM)
